# revision 33
# baseline (speedup 1.0000x reference)
"""LocalSelfAttention (block-diagonal, block=50) Bass kernel for 8 trn2 cores.

Sharding: sequence-parallel over the 41 attention blocks (padded to 48 =
8 cores x 6 blocks). Each core computes QKV projections, block-local
multi-head attention, and the output projection for its 6 blocks (300
tokens). No collectives; the host slices inputs per core and concatenates
the per-core outputs.

All matmul data is fp16 (PSUM accumulation is fp32). Softmax runs without
max-subtraction (logits are O(+-6) for this problem family).

Perf notes (trn2), from trace analysis of previous revisions:
- Weight DMA: 256KB transfers only sustain ~190 GB/s; all inputs are
  host-prepacked into SBUF-image layouts so every transfer is one 0.5-1MB
  contiguous copy (>=341 GB/s). All weight DMAs ride the sync queue;
  the scalar queue carries only x/bias early and y late, so exp/copy work
  on the ACT engine is never stuck behind a weight dma_start's slot-wait.
- Q/K projections run h-major in two 4-head passes (4 PSUM accumulators
  live), consuming weight chunks in DMA arrival order - the PE never
  waits for a full 4MB weight group.
- Softmax normalization is fused into the A^T transpose: instead of 96
  DVE tensor_scalar multiplies (~24us serial DVE that gated the V phase
  through PSUM-ring waits), each head's transpose is a plain matmul
  against a diagonal matrix D = ident * recip built by one tensor_scalar
  on the (otherwise idle) GPSIMD engine: out[k,q] = sum_q' A[q',k]D[q',q]
  = A^T[k,q] * recip[q].
- Transpose units are interleaved into the following projection pass /
  V chains so their PSUM-slot waits hide under projection matmuls.
- PSUM: tag "ps" = 6-slot FIFO ring (accumulators, S tiles, V/Y tiles,
  AV parity pair); tag "atp" = 2 slots for transpose outputs. 8 banks
  total; allocation order = consumption order.
- The output projection computes Y^T (tokens on the moving dim, N=300):
  76.8e3 vs 98.3e3 PE cycles for token-major Y. AV writes OT packed at
  50-column stride so the Y^T rhs is contiguous. Y^T is written fp16.
- Matmuls whose lhsT partition bases differ (row groups 0 vs 64) run
  concurrently inside the PE array and must not target the same PSUM
  bank - AV outputs are split by block parity (opa/opb).
"""

import sys
from contextlib import ExitStack

sys.path.insert(0, "/opt/trn_rl_repo")

import numpy as np

import concourse.bass as bass  # noqa: F401
import concourse.mybir as mybir
import concourse.tile as tile
from concourse import bacc
from concourse.bass_utils import run_bass_kernel_spmd

# ---- problem constants (hardcoded; kernel.py must be self-contained) ----
T, H = 2048, 2048
HEADS, DH = 16, 128
KBLK = 50          # attention block size (tokens)
NEG = -1e9
NCORES = 8
P = T + (KBLK - T % KBLK)          # padded seq len = 2050
NB = P // KBLK                     # 41 real blocks
NB_CORE = 6                        # key/value blocks per core: 5 own + block 40
BPAD = 64                          # per-block padded rows for V (64-aligned)
TPAD = NB_CORE * BPAD              # 384 padded tokens per core (V path)
SCALE = DH ** -0.5
NH_T = H // 128                    # 16 h-tiles of 128
TPACK = NB_CORE * KBLK             # 300 packed key tokens (K/V)
# query-split: each core owns 5 full blocks of queries plus a 1/8 slice of
# block 40 (50 = 2*7 + 6*6 queries); Q/OT/Y run at N=257 instead of 300.
QW = [KBLK] * 5 + [7]              # per-block-slot query widths (max)
TQ = sum(QW)                       # 257 packed query tokens (Q/OT/Y)
QCOL = [0, 50, 100, 150, 200, 250]  # packed query column start per slot
TSPAN = 2 * BPAD - 14              # 114: 2-block partition span
F16 = mybir.dt.float16
F32 = mybir.dt.float32

_CACHE = {}


def _build_program():
    nc = bacc.Bacc("TRN2", target_bir_lowering=False, debug=False,
                   num_devices=NCORES)

    # all inputs are host-prepacked SBUF images: [128, ...] fp16, contiguous
    xq_d = nc.dram_tensor("xq", [128, NH_T * TQ], F16, kind="ExternalInput").ap()
    xk_d = nc.dram_tensor("xk", [128, NH_T * TPACK], F16, kind="ExternalInput").ap()
    xv_d = nc.dram_tensor("xv", [128, NH_T * TPAD], F16, kind="ExternalInput").ap()
    wq_d = nc.dram_tensor("wq", [128, 2 * NH_T * 1024], F16, kind="ExternalInput").ap()
    wk_d = nc.dram_tensor("wk", [128, 2 * NH_T * 1024], F16, kind="ExternalInput").ap()
    wv_d = nc.dram_tensor("wv", [128, 2 * NH_T * 1024], F16, kind="ExternalInput").ap()
    wo_d = nc.dram_tensor("wo", [128, HEADS * 2048], F16, kind="ExternalInput").ap()
    bias_d = nc.dram_tensor("bias", [KBLK, NB_CORE * HEADS * KBLK], F16,
                            kind="ExternalInput").ap()
    ident_d = nc.dram_tensor("ident", [128, 128], F16, kind="ExternalInput").ap()
    # packed Y^T: segment ho holds y[:, 128*ho:128*(ho+1)].T as [128, 257]
    y_d = nc.dram_tensor("y", [128, NH_T * TQ], F16, kind="ExternalOutput").ap()

    with tile.TileContext(nc) as tc, ExitStack() as ctx:
        _emit_body(nc, tc, ctx, xq_d, xk_d, xv_d, wq_d, wk_d, wv_d, wo_d,
                   bias_d, ident_d, y_d)

    nc.compile()
    return nc


def _emit_body(nc, tc, ctx, xq_d, xk_d, xv_d, wq_d, wk_d, wv_d, wo_d, bias_d,
               ident_d, y_d):
    sb = ctx.enter_context(tc.tile_pool(name="persist", bufs=1))
    wring = ctx.enter_context(tc.tile_pool(name="wring", bufs=13))
    apool = ctx.enter_context(tc.tile_pool(name="apool", bufs=4))
    dpool = ctx.enter_context(tc.tile_pool(name="dpool", bufs=26))
    ps = ctx.enter_context(tc.tile_pool(name="ps", bufs=6, space="PSUM"))

    def pst(shape, dtype, name):
        return ps.tile(shape, dtype, tag="ps", name=name)

    # ---- persistent SBUF arrays ----
    xq = sb.tile([128, NH_T * TQ], F16, tag="xq")
    xk = sb.tile([128, NH_T * TPACK], F16, tag="xk")
    xv = sb.tile([128, NH_T * TPAD], F16, tag="xv")
    # qt/kt hold one 8-head group at a time (reused across the 2 groups)
    qt = [sb.tile([128, TQ], F16, tag=f"qt{e}", name=f"qt{e}") for e in range(8)]
    kt = [sb.tile([128, TPACK], F16, tag=f"kt{e}", name=f"kt{e}") for e in range(8)]
    ot = [sb.tile([128, TPACK], F16, tag=f"ot{e}", name=f"ot{e}")
          for e in range(HEADS)]   # only [0:TQ] is read by the Y^T projection
    vsb = [sb.tile([128, H], F16, tag=f"v{t}", name=f"vsb{t}") for t in range(3)]
    atb = [sb.tile([128, HEADS * KBLK], F16, tag=f"at{b}", name=f"atb{b}")
           for b in range(NB_CORE)]
    bias_sb = sb.tile([KBLK, NB_CORE * HEADS * KBLK], F16, tag="bias")
    ident = sb.tile([128, 128], F16, tag="ident")

    def wchunk(w_hbm, g, h0, nh, eng, wmap):
        # one chunk = nh h-tiles of a 4MB weight group, on the given queue
        wt = wring.tile([128, nh * 1024], F16, tag="w", name="w",
                        padded_shape=[128, 4096])
        base = NH_T * 1024 * g + 1024 * h0
        eng.dma_start(wt[:], w_hbm[:, base:base + nh * 1024])
        for i in range(nh):
            wmap.append((wt, 1024 * i))
        return wmap

    def wchunks(w_hbm, g, hsplits, engs=None):
        wmap = []
        h0 = 0
        for i, nh in enumerate(hsplits):
            eng = engs[i] if engs else nc.sync
            wchunk(w_hbm, g, h0, nh, eng, wmap)
            h0 += nh
        return wmap

    def proj_qk(wmap, src, tl, dst, mids=()):
        # h-major in a 6-head + 2-head pass: consume weight chunks in DMA
        # arrival order. Pass A's PE time (~12.2us) matches one group's DMA
        # (~11.7us) so the PE is never starved; pass B re-reads from SBUF
        # while the next group's DMA runs ahead. `mids`: emit-units (1 per
        # h-step) whose PSUM-slot waits hide under projection matmuls.
        mids = list(mids)
        for half, (e0, ne) in enumerate(((0, 6), (6, 2))):
            acc = [pst([128, tl], F32, f"acc{e}") for e in range(ne)]
            for h in range(NH_T):
                wsb, coff = wmap[h]
                for e in range(ne):
                    el = e0 + e
                    nc.tensor.matmul(acc[e][:],
                                     wsb[:, coff + 128 * el:coff + 128 * (el + 1)],
                                     src[:, tl * h:tl * (h + 1)],
                                     start=(h == 0), stop=(h == NH_T - 1))
                if mids and (half == 1 or h >= 4):
                    mids.pop(0)()
            for e in range(ne):
                nc.vector.tensor_copy(dst[e0 + e][:], acc[e][:])
        while mids:
            mids.pop(0)()

    asb_live = {}

    def attention_softmax(g):
        # S matmuls + softmax for heads 8g..8g+8 of every block. Blocks are
        # processed in pairs sharing [128, .] tiles at partition bases 0/64.
        # The additive bias is injected by an identity-matmul accumulation
        # (start=True) BEFORE the S matmuls; the softmax scale rides on exp's
        # free affine (bias pre-scaled by sqrt(DH) on the host). Instead of
        # normalizing A on the DVE, build per-head diagonal reciprocal
        # matrices D (on GPSIMD); the transpose matmul then applies them.
        for bp in range(NB_CORE // 2):
            asb = apool.tile([128, 8 * BPAD], F16, tag="a_exp", name="asb")
            # epsilon (not 0): gap/pad rows must keep sums > 0 so recip stays
            # finite - D's off-diagonal zeros would otherwise become 0*inf=NaN
            nc.vector.memset(asb[:], 1e-4)
            sums = apool.tile([128, 8], F32, tag="sums", name="sums")
            recip = apool.tile([128, 8], F32, tag="recip", name="recip")
            for par in range(2):
                b = 2 * bp + par
                pb = BPAD * par
                tcol = KBLK * b
                qcol, qw = QCOL[b], QW[b]
                sp = pst([KBLK, 8 * KBLK], F32, "sp")
                boff = (b * 2 + g) * 8 * KBLK
                nc.tensor.matmul(sp[:], ident[0:KBLK, 0:KBLK],
                                 bias_sb[:, boff:boff + 8 * KBLK],
                                 start=True, stop=False)
                for el in range(8):
                    nc.tensor.matmul(sp[0:qw, KBLK * el:KBLK * (el + 1)],
                                     qt[el][:, qcol:qcol + qw],
                                     kt[el][:, tcol:tcol + KBLK],
                                     start=False, stop=(el == 7))
                nc.scalar.activation(
                    asb[pb:pb + KBLK, :].rearrange("p (e x) -> p e x", e=8)[:, :, 0:KBLK],
                    sp[:], mybir.ActivationFunctionType.Exp, scale=SCALE)
            nc.vector.reduce_sum(
                sums[:], asb.rearrange("p (e x) -> p e x", e=8)[:, :, 0:KBLK],
                axis=mybir.AxisListType.X)
            nc.vector.reciprocal(recip[:], sums[:])
            ds = []
            for el in range(8):
                dt_ = dpool.tile([128, TSPAN], F16, tag="d", name="dt")
                nc.vector.tensor_scalar_mul(
                    dt_[0:TSPAN, :], ident[0:TSPAN, 0:TSPAN],
                    recip[0:TSPAN, el:el + 1])
                ds.append(dt_)
            asb_live[(bp, g)] = (asb, ds)

    def transpose_units(g):
        # one unit per (el, bp): a [50,114] matmul A^T*D + 2 quadrant copies.
        # el-outer so head e's A^T completes after ~3(el+1) units - the AV
        # matmuls interleaved into the V phase depend on it in head order.
        pairs = [asb_live.pop((bp, g)) for bp in range(NB_CORE // 2)]
        units = []
        for el in range(8):
            for bp in range(NB_CORE // 2):
                asb, ds = pairs[bp]

                def unit(bp=bp, asb=asb, d=ds[el], el=el):
                    e = 8 * g + el
                    atp = ps.tile([KBLK, TSPAN], F32, tag="atp", name="atp",
                                  bufs=2)
                    nc.tensor.matmul(atp[:],
                                     asb[0:TSPAN, BPAD * el:BPAD * el + KBLK],
                                     d[0:TSPAN, :], start=True, stop=True)
                    for par in range(2):
                        b = 2 * bp + par
                        base = BPAD * par
                        eng = nc.vector.tensor_copy if par == 0 else nc.scalar.copy
                        eng(atb[b][base:base + KBLK, KBLK * e:KBLK * (e + 1)],
                            atp[:, BPAD * par:BPAD * par + KBLK])
                units.append(unit)
        return units

    def av_heads(es):
        # A^T @ V -> OT[dh, t], packed at the QCOL offsets. Matmuls with
        # different lhsT partition bases (row groups 0 vs 64) run concurrently
        # in the PE array and must not share a PSUM bank: one PSUM tile per
        # block parity, then strided copies into ot[e].
        for e in es:
            opa = pst([128, TPACK], F32, "opa")
            opb = pst([128, TPACK], F32, "opb")
            opp = (opa, opb)
            for b in range(NB_CORE):
                par = b % 2
                base = BPAD * par
                qcol, qw = QCOL[b], QW[b]
                nc.tensor.matmul(
                    opp[par][:, qcol:qcol + qw],
                    vsb[b // 2][base:base + KBLK, 128 * e:128 * (e + 1)],
                    atb[b][base:base + KBLK, KBLK * e:KBLK * e + qw],
                    start=True, stop=True)
            eng = nc.scalar.copy if e % 2 == 0 else nc.vector.tensor_copy
            # parity 0: blocks 0,2,4 at columns 100j..100j+50
            src = opa.rearrange("p (j x) -> p j x", j=3)
            dst = ot[e].rearrange("p (j x) -> p j x", j=3)
            eng(dst[:, :, 0:KBLK], src[:, :, 0:KBLK])
            # parity 1: blocks 1,3 at columns 50+100j; partial block at 250
            src = opb[:, KBLK:250].rearrange("p (j x) -> p j x", j=2)
            dst = ot[e][:, KBLK:250].rearrange("p (j x) -> p j x", j=2)
            eng(dst[:, :, 0:KBLK], src[:, :, 0:KBLK])
            eng(ot[e][:, 250:TQ], opb[:, 250:TQ])

    # ================= emission =================
    # The first 13 weight chunks (wq0/wk0/wq1 = the 13-slot ring's first
    # allocations) have no slot-waits, so they may ride either queue; xq
    # pieces and weight chunks are interleaved across both queues in h-
    # consumption order. From wk1 on, weights go sync-only so the scalar
    # engine (exps, copies) never blocks on a slot-gated dma_start.
    sy, sc = nc.sync, nc.scalar

    def xqp(h0, h1):
        sc.dma_start(xq[:, h0 * TQ:h1 * TQ], xq_d[:, h0 * TQ:h1 * TQ])

    # per-queue delivery tracks per-h consumption order (h needs both its
    # xq piece and its weight chunk); the first xq piece leads the sync
    # queue - the scalar queue's first transfer starts ~3us later
    sy.dma_start(xq[:, 0:2 * TQ], xq_d[:, 0:2 * TQ])
    wm_q0 = wchunks(wq_d, 0, [2], engs=[sy])                        # h0-1
    wchunk(wq_d, 0, 2, 2, sc, wm_q0)                                # h2-3
    xqp(2, 4)
    wchunk(wq_d, 0, 4, 4, sy, wm_q0)                                # h4-7
    xqp(4, 8)
    wchunk(wq_d, 0, 8, 4, sc, wm_q0)                                # h8-11
    xqp(8, 16)
    wchunk(wq_d, 0, 12, 4, sy, wm_q0)                               # h12-15
    sy.dma_start(xk[:], xk_d)
    sc.dma_start(bias_sb[:], bias_d)
    sc.dma_start(ident[:], ident_d)
    wm_k0 = wchunks(wk_d, 0, [4, 4, 4, 4], engs=[sc, sy, sc, sy])
    wm_q1 = wchunks(wq_d, 1, [4, 4, 4, 4], engs=[sc, sy, sc, sy])
    proj_qk(wm_q0, xq, TQ, qt)
    wm_k1 = wchunks(wk_d, 1, [4, 4, 4, 4])
    proj_qk(wm_k0, xk, TPACK, kt)
    attention_softmax(0)
    nc.scalar.dma_start(xv[:], xv_d)
    wm_v0 = wchunks(wv_d, 0, [4, 4, 4, 4])
    proj_qk(wm_q1, xq, TQ, qt, mids=transpose_units(0))
    wm_v1 = wchunks(wv_d, 1, [4, 4, 4, 4])
    proj_qk(wm_k1, xk, TPACK, kt)
    attention_softmax(1)

    # stage all of Wo now; arrives well before the output projection
    wt_wo = []
    for c in range(8):
        w = wring.tile([128, 4096], F16, tag="w", name="w")
        nc.sync.dma_start(w[:], wo_d[:, 4096 * c:4096 * (c + 1)])
        wt_wo.append(w)

    # ---- V projection, token-major: out[t, ed] = xT[h, t].T @ W[h, ed] ----
    # T1 units interleave into the first V groups; AV head group 4(eg-1)..4eg
    # follows V group eg (its vsb columns and A^T tiles are ready), so only
    # AV e12-15 remains after V and the Y^T projection starts earlier.
    t1units = transpose_units(1)
    for g in range(2):
        wm = (wm_v0, wm_v1)[g]
        for eo in range(2):
            eg = 2 * g + eo
            for tt in range(3):            # token tiles of 128
                pt = pst([128, 512], F32, "vp")
                for h in range(NH_T):
                    wsb, coff = wm[h]
                    nc.tensor.matmul(pt[:],
                                     xv[:, TPAD * h + 128 * tt:TPAD * h + 128 * (tt + 1)],
                                     wsb[:, coff + 512 * eo:coff + 512 * (eo + 1)],
                                     start=(h == 0), stop=(h == NH_T - 1))
                nc.scalar.copy(vsb[tt][:, 512 * eg:512 * (eg + 1)], pt[:])
                for _ in range(4):
                    if t1units:
                        t1units.pop(0)()
            if eg >= 1:
                av_heads(range(4 * (eg - 1), 4 * eg))
    while t1units:
        t1units.pop(0)()
    av_heads(range(12, HEADS))

    # ---- output projection: yT[h, t] = sum_e Wo[ed_e, h].T @ OT[ed_e, t] ----
    for ho in range(NH_T):
        yp = pst([128, TQ], F32, "yp")
        for e in range(HEADS):
            wsb = wt_wo[e // 2]
            coff = 2048 * (e % 2)
            nc.tensor.matmul(yp[:],
                             wsb[:, coff + 128 * ho:coff + 128 * (ho + 1)],
                             ot[e][:, 0:TQ],
                             start=(e == 0), stop=(e == HEADS - 1))
        ysb = apool.tile([128, TQ], F16, tag="y", name="ysb")
        nc.vector.tensor_copy(ysb[:], yp[:])
        (sy if ho >= 14 and ho % 2 == 0 else sc).dma_start(
            y_d[:, TQ * ho:TQ * (ho + 1)], ysb[:])


def _prep_inputs(x, mask, bias, Wq, Wk, Wv, Wo):
    """Slice/pad/transpose the full inputs into per-core prepacked maps."""
    x = np.asarray(x, np.float32).reshape(T, H)
    mask = np.asarray(mask, np.float32).reshape(T)
    bias = np.asarray(bias, np.float32).reshape(HEADS, T, T)

    # padded x (tokens) and mask, as in the reference
    xp = np.zeros((P, H), np.float32)
    xp[:T] = x
    mb = np.full(P, NEG, np.float32)
    mb[:T] = mask

    def mk_bias(qlo, qn_, klo):
        # [HEADS, KBLK, KBLK] combined bias for queries [qlo, qlo+qn_) vs
        # keys [klo, klo+KBLK). Rows >= qn_ fully masked. Pre-scaled by
        # sqrt(DH) (exp applies scale=DH**-0.5); masked entries use -100
        # (* SCALE -> -8.8, exp -> 1.4e-4: negligible vs real weights but
        # NONZERO so junk/pad row-sums stay finite - 1/0 = inf would turn
        # D's off-diagonal zeros into 0*inf = NaN, see _emit_body)
        blk = np.full((HEADS, KBLK, KBLK), -100.0, np.float32)
        qr = min(qn_, max(0, T - qlo))
        kr = min(KBLK, max(0, T - klo))
        if qr > 0 and kr > 0:
            b_ = bias[:, qlo:qlo + qr, klo:klo + kr] * (DH ** 0.5)
            pair = mb[qlo:qlo + qr, None] * mb[None, klo:klo + kr]
            blk[:, :qr, :kr] = b_ + np.where(pair > 0, 0.0, -100.0)[None]
        return blk

    # weight SBUF images: [128, 2g * 16h * (8e*128d)] for wq/wk/wv
    def pack_w(W):
        w = np.ascontiguousarray(W.reshape(H, H), np.float32).astype(np.float16)
        w = w.reshape(NH_T, 128, 2, 8 * 128)          # [h, p, g, ed]
        w = w.transpose(1, 2, 0, 3)                   # [p, g, h, ed]
        return np.ascontiguousarray(w.reshape(128, 2 * NH_T * 1024))

    wq = pack_w(Wq)
    wk = pack_w(Wk)
    wv = pack_w(Wv)
    # wo image: [128d, 16e * 2048h]
    wo = np.ascontiguousarray(Wo.reshape(HEADS, DH, H), np.float32).astype(np.float16)
    wo = wo.transpose(1, 0, 2)                        # [d, e, h]
    wo = np.ascontiguousarray(wo.reshape(128, HEADS * 2048))
    ident = np.eye(128, dtype=np.float16)

    def pack_x(xm, tl):
        xt = xm.T.astype(np.float16)              # [H, tl]
        xt = xt.reshape(NH_T, 128, tl).transpose(1, 0, 2)
        return np.ascontiguousarray(xt.reshape(128, NH_T * tl))

    in_maps = []
    for c in range(NCORES):
        qn = 7 if c < 2 else 6
        qoff = 7 * c if c < 2 else 14 + 6 * (c - 2)
        kblocks = [5 * c + j for j in range(5)] + [NB - 1]
        # query tokens: 5 own blocks + a qn-slice of block 40
        xqm = np.zeros((TQ, H), np.float32)
        # key/value tokens: the 6 kblocks (xk packed at 50, xv padded to 64)
        xkm = np.zeros((TPACK, H), np.float32)
        xvm = np.zeros((TPAD, H), np.float32)
        comb = np.zeros((NB_CORE, HEADS, KBLK, KBLK), np.float32)
        for j, b in enumerate(kblocks):
            lo = KBLK * b
            n = min(KBLK, max(0, T - lo))
            xkm[KBLK * j:KBLK * j + n] = xp[lo:lo + n]
            xvm[BPAD * j:BPAD * j + n] = xp[lo:lo + n]
            if j < 5:
                xqm[QCOL[j]:QCOL[j] + n] = xp[lo:lo + n]
                comb[j] = mk_bias(lo, KBLK, lo)
            else:
                qlo = KBLK * (NB - 1) + qoff
                nq = min(qn, max(0, T - qlo))
                xqm[QCOL[5]:QCOL[5] + nq] = xp[qlo:qlo + nq]
                comb[j] = mk_bias(qlo, qn, lo)

        # bias layout: [q, (block, headgroup, head, k)], fp16
        bc = comb.transpose(2, 0, 1, 3).reshape(KBLK, NB_CORE * HEADS * KBLK)
        in_maps.append({
            "xq": pack_x(xqm, TQ),
            "xk": pack_x(xkm, TPACK),
            "xv": pack_x(xvm, TPAD),
            "wq": wq, "wk": wk, "wv": wv, "wo": wo,
            "bias": np.ascontiguousarray(bc.astype(np.float16)),
            "ident": ident,
        })
    return in_maps


def _gather(results):
    out = np.empty((T, H), np.float32)
    for c in range(NCORES):
        # y is packed Y^T: [128, 16ho * 257] fp16
        yc = results[c]["y"].astype(np.float32)
        yc = yc.reshape(128, NH_T, TQ).transpose(1, 0, 2).reshape(H, TQ)
        for j in range(5):
            lo = KBLK * (5 * c + j)
            out[lo:lo + KBLK] = yc[:, QCOL[j]:QCOL[j] + KBLK].T
        qn = 7 if c < 2 else 6
        qoff = 7 * c if c < 2 else 14 + 6 * (c - 2)
        qlo = KBLK * (NB - 1) + qoff
        nq = min(qn, max(0, T - qlo))
        if nq > 0:
            out[qlo:qlo + nq] = yc[:, QCOL[5]:QCOL[5] + nq].T
    return out


def run(trace=False, **inputs):
    if "nc" not in _CACHE:
        _CACHE["nc"] = _build_program()
    nc = _CACHE["nc"]
    in_maps = _prep_inputs(
        inputs["x_BxTxH"], inputs["mask_BxT"], inputs["attention_bias_BxHxTxT"],
        inputs["Wq"], inputs["Wk"], inputs["Wv"], inputs["Wo"])
    res = run_bass_kernel_spmd(nc, in_maps, list(range(NCORES)), trace=trace)
    out = _gather(res.results)[None]       # restore batch dim [1, T, H]
    return out, res.exec_time_ns


def kernel(**inputs):
    out, _ = run(trace=False, **inputs)
    return out


# revision 34
# speedup vs baseline: 1.0197x; 1.0197x over previous
"""LocalSelfAttention (block-diagonal, block=50) Bass kernel for 8 trn2 cores.

Sharding: sequence-parallel over the 41 attention blocks (padded to 48 =
8 cores x 6 blocks). Each core computes QKV projections, block-local
multi-head attention, and the output projection for its 6 blocks (300
tokens). No collectives; the host slices inputs per core and concatenates
the per-core outputs.

All matmul data is fp16 (PSUM accumulation is fp32). Softmax runs without
max-subtraction (logits are O(+-6) for this problem family).

Perf notes (trn2), from trace analysis of previous revisions:
- Weight DMA: 256KB transfers only sustain ~190 GB/s; all inputs are
  host-prepacked into SBUF-image layouts so every transfer is one 0.5-1MB
  contiguous copy (>=341 GB/s). All weight DMAs ride the sync queue;
  the scalar queue carries only x/bias early and y late, so exp/copy work
  on the ACT engine is never stuck behind a weight dma_start's slot-wait.
- Q/K projections run h-major in two 4-head passes (4 PSUM accumulators
  live), consuming weight chunks in DMA arrival order - the PE never
  waits for a full 4MB weight group.
- Softmax normalization is fused into the A^T transpose: instead of 96
  DVE tensor_scalar multiplies (~24us serial DVE that gated the V phase
  through PSUM-ring waits), each head's transpose is a plain matmul
  against a diagonal matrix D = ident * recip built by one tensor_scalar
  on the (otherwise idle) GPSIMD engine: out[k,q] = sum_q' A[q',k]D[q',q]
  = A^T[k,q] * recip[q].
- Transpose units are interleaved into the following projection pass /
  V chains so their PSUM-slot waits hide under projection matmuls.
- PSUM: tag "ps" = 6-slot FIFO ring (accumulators, S tiles, V/Y tiles,
  AV parity pair); tag "atp" = 2 slots for transpose outputs. 8 banks
  total; allocation order = consumption order.
- The output projection computes Y^T (tokens on the moving dim, N=300):
  76.8e3 vs 98.3e3 PE cycles for token-major Y. AV writes OT packed at
  50-column stride so the Y^T rhs is contiguous. Y^T is written fp16.
- Matmuls whose lhsT partition bases differ (row groups 0 vs 64) run
  concurrently inside the PE array and must not target the same PSUM
  bank - AV outputs are split by block parity (opa/opb).
"""

import sys
from contextlib import ExitStack

sys.path.insert(0, "/opt/trn_rl_repo")

import numpy as np

import concourse.bass as bass  # noqa: F401
import concourse.mybir as mybir
import concourse.tile as tile
from concourse import bacc
from concourse.bass_utils import run_bass_kernel_spmd

# ---- problem constants (hardcoded; kernel.py must be self-contained) ----
T, H = 2048, 2048
HEADS, DH = 16, 128
KBLK = 50          # attention block size (tokens)
NEG = -1e9
NCORES = 8
P = T + (KBLK - T % KBLK)          # padded seq len = 2050
NB = P // KBLK                     # 41 real blocks
NB_CORE = 6                        # key/value blocks per core: 5 own + block 40
BPAD = 64                          # per-block padded rows for V (64-aligned)
TPAD = NB_CORE * BPAD              # 384 padded tokens per core (V path)
SCALE = DH ** -0.5
NH_T = H // 128                    # 16 h-tiles of 128
TPACK = NB_CORE * KBLK             # 300 packed key tokens (K/V)
# query-split: each core owns 5 full blocks of queries plus a 1/8 slice of
# block 40 (50 = 2*7 + 6*6 queries); Q/OT/Y run at N=257 instead of 300.
QW = [KBLK] * 5 + [7]              # per-block-slot query widths (max)
TQ = sum(QW)                       # 257 packed query tokens (Q/OT/Y)
QCOL = [0, 50, 100, 150, 200, 250]  # packed query column start per slot
TSPAN = 2 * BPAD - 14              # 114: 2-block partition span
F16 = mybir.dt.float16
F32 = mybir.dt.float32

_CACHE = {}


def _build_program():
    nc = bacc.Bacc("TRN2", target_bir_lowering=False, debug=False,
                   num_devices=NCORES)

    # all inputs are host-prepacked SBUF images: [128, ...] fp16, contiguous
    xq_d = nc.dram_tensor("xq", [128, NH_T * TQ], F16, kind="ExternalInput").ap()
    xk_d = nc.dram_tensor("xk", [128, NH_T * TPACK], F16, kind="ExternalInput").ap()
    xv_d = nc.dram_tensor("xv", [128, NH_T * TPAD], F16, kind="ExternalInput").ap()
    wq_d = nc.dram_tensor("wq", [128, 2 * NH_T * 1024], F16, kind="ExternalInput").ap()
    wk_d = nc.dram_tensor("wk", [128, 2 * NH_T * 1024], F16, kind="ExternalInput").ap()
    wv_d = nc.dram_tensor("wv", [128, 2 * NH_T * 1024], F16, kind="ExternalInput").ap()
    wo_d = nc.dram_tensor("wo", [128, HEADS * 2048], F16, kind="ExternalInput").ap()
    bias_d = nc.dram_tensor("bias", [KBLK, NB_CORE * HEADS * KBLK], F16,
                            kind="ExternalInput").ap()
    ident_d = nc.dram_tensor("ident", [128, 128], F16, kind="ExternalInput").ap()
    # packed Y^T: segment ho holds y[:, 128*ho:128*(ho+1)].T as [128, 257]
    y_d = nc.dram_tensor("y", [128, NH_T * TQ], F16, kind="ExternalOutput").ap()

    with tile.TileContext(nc) as tc, ExitStack() as ctx:
        _emit_body(nc, tc, ctx, xq_d, xk_d, xv_d, wq_d, wk_d, wv_d, wo_d,
                   bias_d, ident_d, y_d)

    nc.compile()
    return nc


def _emit_body(nc, tc, ctx, xq_d, xk_d, xv_d, wq_d, wk_d, wv_d, wo_d, bias_d,
               ident_d, y_d):
    sb = ctx.enter_context(tc.tile_pool(name="persist", bufs=1))
    wring = ctx.enter_context(tc.tile_pool(name="wring", bufs=13))
    apool = ctx.enter_context(tc.tile_pool(name="apool", bufs=4))
    dpool = ctx.enter_context(tc.tile_pool(name="dpool", bufs=26))
    ps = ctx.enter_context(tc.tile_pool(name="ps", bufs=6, space="PSUM"))

    def pst(shape, dtype, name):
        return ps.tile(shape, dtype, tag="ps", name=name)

    # ---- persistent SBUF arrays ----
    xq = sb.tile([128, NH_T * TQ], F16, tag="xq")
    xk = sb.tile([128, NH_T * TPACK], F16, tag="xk")
    xv = sb.tile([128, NH_T * TPAD], F16, tag="xv")
    # qt/kt hold one 8-head group at a time (reused across the 2 groups)
    qt = [sb.tile([128, TQ], F16, tag=f"qt{e}", name=f"qt{e}") for e in range(8)]
    kt = [sb.tile([128, TPACK], F16, tag=f"kt{e}", name=f"kt{e}") for e in range(8)]
    ot = [sb.tile([128, TPACK], F16, tag=f"ot{e}", name=f"ot{e}")
          for e in range(HEADS)]   # only [0:TQ] is read by the Y^T projection
    vsb = [sb.tile([128, H], F16, tag=f"v{t}", name=f"vsb{t}") for t in range(3)]
    atb = [sb.tile([128, HEADS * KBLK], F16, tag=f"at{b}", name=f"atb{b}")
           for b in range(NB_CORE)]
    bias_sb = sb.tile([KBLK, NB_CORE * HEADS * KBLK], F16, tag="bias")
    ident = sb.tile([128, 128], F16, tag="ident")

    def wchunk(w_hbm, g, h0, nh, eng, wmap):
        # one chunk = nh h-tiles of a 4MB weight group, on the given queue
        wt = wring.tile([128, nh * 1024], F16, tag="w", name="w",
                        padded_shape=[128, 4096])
        base = NH_T * 1024 * g + 1024 * h0
        eng.dma_start(wt[:], w_hbm[:, base:base + nh * 1024])
        for i in range(nh):
            wmap.append((wt, 1024 * i))
        return wmap

    def wchunks(w_hbm, g, hsplits, engs=None):
        wmap = []
        h0 = 0
        for i, nh in enumerate(hsplits):
            eng = engs[i] if engs else nc.sync
            wchunk(w_hbm, g, h0, nh, eng, wmap)
            h0 += nh
        return wmap

    def proj_qk(wmap, src, tl, dst, mids=()):
        # h-major in a 6-head + 2-head pass: consume weight chunks in DMA
        # arrival order. Pass A's PE time (~12.2us) matches one group's DMA
        # (~11.7us) so the PE is never starved; pass B re-reads from SBUF
        # while the next group's DMA runs ahead. `mids`: emit-units (1 per
        # h-step) whose PSUM-slot waits hide under projection matmuls.
        mids = list(mids)
        for half, (e0, ne) in enumerate(((0, 6), (6, 2))):
            acc = [pst([128, tl], F32, f"acc{e}") for e in range(ne)]
            for h in range(NH_T):
                wsb, coff = wmap[h]
                for e in range(ne):
                    el = e0 + e
                    nc.tensor.matmul(acc[e][:],
                                     wsb[:, coff + 128 * el:coff + 128 * (el + 1)],
                                     src[:, tl * h:tl * (h + 1)],
                                     start=(h == 0), stop=(h == NH_T - 1))
                if mids and (half == 1 or h >= 4):
                    mids.pop(0)()
            for e in range(ne):
                nc.vector.tensor_copy(dst[e0 + e][:], acc[e][:])
        while mids:
            mids.pop(0)()

    asb_live = {}

    def attention_softmax(g):
        # S matmuls + softmax for heads 8g..8g+8 of every block. Blocks are
        # processed in pairs sharing [128, .] tiles at partition bases 0/64.
        # The additive bias is injected by an identity-matmul accumulation
        # (start=True) BEFORE the S matmuls; the softmax scale rides on exp's
        # free affine (bias pre-scaled by sqrt(DH) on the host). Instead of
        # normalizing A on the DVE, build per-head diagonal reciprocal
        # matrices D (on GPSIMD); the transpose matmul then applies them.
        for bp in range(NB_CORE // 2):
            asb = apool.tile([128, 8 * BPAD], F16, tag="a_exp", name="asb")
            # epsilon (not 0): gap/pad rows must keep sums > 0 so recip stays
            # finite - D's off-diagonal zeros would otherwise become 0*inf=NaN
            nc.vector.memset(asb[:], 1e-4)
            sums = apool.tile([128, 8], F32, tag="sums", name="sums")
            recip = apool.tile([128, 8], F32, tag="recip", name="recip")
            for par in range(2):
                b = 2 * bp + par
                pb = BPAD * par
                tcol = KBLK * b
                qcol, qw = QCOL[b], QW[b]
                sp = pst([KBLK, 8 * KBLK], F32, "sp")
                boff = (b * 2 + g) * 8 * KBLK
                nc.tensor.matmul(sp[:], ident[0:KBLK, 0:KBLK],
                                 bias_sb[:, boff:boff + 8 * KBLK],
                                 start=True, stop=False)
                for el in range(8):
                    nc.tensor.matmul(sp[0:qw, KBLK * el:KBLK * (el + 1)],
                                     qt[el][:, qcol:qcol + qw],
                                     kt[el][:, tcol:tcol + KBLK],
                                     start=False, stop=(el == 7))
                nc.scalar.activation(
                    asb[pb:pb + KBLK, :].rearrange("p (e x) -> p e x", e=8)[:, :, 0:KBLK],
                    sp[:], mybir.ActivationFunctionType.Exp, scale=SCALE)
            nc.vector.reduce_sum(
                sums[:], asb.rearrange("p (e x) -> p e x", e=8)[:, :, 0:KBLK],
                axis=mybir.AxisListType.X)
            nc.vector.reciprocal(recip[:], sums[:])
            ds = []
            for el in range(8):
                dt_ = dpool.tile([128, TSPAN], F16, tag="d", name="dt")
                nc.vector.tensor_scalar_mul(
                    dt_[0:TSPAN, :], ident[0:TSPAN, 0:TSPAN],
                    recip[0:TSPAN, el:el + 1])
                ds.append(dt_)
            asb_live[(bp, g)] = (asb, ds)

    def transpose_units(g):
        # one unit per (el, bp): a [50,114] matmul A^T*D + 2 quadrant copies.
        # el-outer so head e's A^T completes after ~3(el+1) units - the AV
        # matmuls interleaved into the V phase depend on it in head order.
        pairs = [asb_live.pop((bp, g)) for bp in range(NB_CORE // 2)]
        units = []
        for el in range(8):
            for bp in range(NB_CORE // 2):
                asb, ds = pairs[bp]

                def unit(bp=bp, asb=asb, d=ds[el], el=el):
                    e = 8 * g + el
                    atp = ps.tile([KBLK, TSPAN], F32, tag="atp", name="atp",
                                  bufs=2)
                    nc.tensor.matmul(atp[:],
                                     asb[0:TSPAN, BPAD * el:BPAD * el + KBLK],
                                     d[0:TSPAN, :], start=True, stop=True)
                    for par in range(2):
                        b = 2 * bp + par
                        base = BPAD * par
                        eng = nc.vector.tensor_copy if par == 0 else nc.scalar.copy
                        eng(atb[b][base:base + KBLK, KBLK * e:KBLK * (e + 1)],
                            atp[:, BPAD * par:BPAD * par + KBLK])
                units.append(unit)
        return units

    def av_heads(es):
        # A^T @ V -> OT[dh, t], packed at the QCOL offsets. Matmuls with
        # different lhsT partition bases (row groups 0 vs 64) run concurrently
        # in the PE array and must not share a PSUM bank: one PSUM tile per
        # block parity, then strided copies into ot[e].
        for e in es:
            opa = pst([128, TPACK], F32, "opa")
            opb = pst([128, TPACK], F32, "opb")
            opp = (opa, opb)
            for b in range(NB_CORE):
                par = b % 2
                base = BPAD * par
                qcol, qw = QCOL[b], QW[b]
                nc.tensor.matmul(
                    opp[par][:, qcol:qcol + qw],
                    vsb[b // 2][base:base + KBLK, 128 * e:128 * (e + 1)],
                    atb[b][base:base + KBLK, KBLK * e:KBLK * e + qw],
                    start=True, stop=True)
            eng = nc.scalar.copy if e % 2 == 0 else nc.vector.tensor_copy
            # parity 0: blocks 0,2,4 at columns 100j..100j+50
            src = opa.rearrange("p (j x) -> p j x", j=3)
            dst = ot[e].rearrange("p (j x) -> p j x", j=3)
            eng(dst[:, :, 0:KBLK], src[:, :, 0:KBLK])
            # parity 1: blocks 1,3 at columns 50+100j; partial block at 250
            src = opb[:, KBLK:250].rearrange("p (j x) -> p j x", j=2)
            dst = ot[e][:, KBLK:250].rearrange("p (j x) -> p j x", j=2)
            eng(dst[:, :, 0:KBLK], src[:, :, 0:KBLK])
            eng(ot[e][:, 250:TQ], opb[:, 250:TQ])

    # ================= emission =================
    # The first 13 weight chunks (wq0/wk0/wq1 = the 13-slot ring's first
    # allocations) have no slot-waits, so they may ride either queue; xq
    # pieces and weight chunks are interleaved across both queues in h-
    # consumption order. From wk1 on, weights go sync-only so the scalar
    # engine (exps, copies) never blocks on a slot-gated dma_start.
    sy, sc = nc.sync, nc.scalar

    def xqp(h0, h1):
        sc.dma_start(xq[:, h0 * TQ:h1 * TQ], xq_d[:, h0 * TQ:h1 * TQ])

    # per-queue delivery tracks per-h consumption order (h needs both its
    # xq piece on scalar and its weight chunk on either queue)
    xqp(0, 2)
    wm_q0 = wchunks(wq_d, 0, [2], engs=[sy])                        # h0-1
    wchunk(wq_d, 0, 2, 2, sc, wm_q0)                                # h2-3
    xqp(2, 4)
    wchunk(wq_d, 0, 4, 4, sy, wm_q0)                                # h4-7
    xqp(4, 8)
    wchunk(wq_d, 0, 8, 4, sc, wm_q0)                                # h8-11
    xqp(8, 16)
    wchunk(wq_d, 0, 12, 4, sy, wm_q0)                               # h12-15
    sy.dma_start(xk[:], xk_d)
    sc.dma_start(bias_sb[:], bias_d)
    sc.dma_start(ident[:], ident_d)
    wm_k0 = wchunks(wk_d, 0, [4, 4, 4, 4], engs=[sc, sy, sc, sy])
    wm_q1 = wchunks(wq_d, 1, [4, 4, 4, 4], engs=[sc, sy, sc, sy])
    proj_qk(wm_q0, xq, TQ, qt)
    wm_k1 = wchunks(wk_d, 1, [4, 4, 4, 4])
    proj_qk(wm_k0, xk, TPACK, kt)
    attention_softmax(0)
    nc.scalar.dma_start(xv[:], xv_d)
    wm_v0 = wchunks(wv_d, 0, [4, 4, 4, 4])
    proj_qk(wm_q1, xq, TQ, qt, mids=transpose_units(0))
    wm_v1 = wchunks(wv_d, 1, [4, 4, 4, 4])
    proj_qk(wm_k1, xk, TPACK, kt)
    attention_softmax(1)

    # stage all of Wo now; arrives well before the output projection
    wt_wo = []
    for c in range(8):
        w = wring.tile([128, 4096], F16, tag="w", name="w")
        nc.sync.dma_start(w[:], wo_d[:, 4096 * c:4096 * (c + 1)])
        wt_wo.append(w)

    # ---- V projection, token-major: out[t, ed] = xT[h, t].T @ W[h, ed] ----
    # T1 units interleave into the first V groups; AV head group 4(eg-1)..4eg
    # follows V group eg (its vsb columns and A^T tiles are ready), so only
    # AV e12-15 remains after V and the Y^T projection starts earlier.
    t1units = transpose_units(1)
    for g in range(2):
        wm = (wm_v0, wm_v1)[g]
        for eo in range(2):
            eg = 2 * g + eo
            for tt in range(3):            # token tiles of 128
                pt = pst([128, 512], F32, "vp")
                for h in range(NH_T):
                    wsb, coff = wm[h]
                    nc.tensor.matmul(pt[:],
                                     xv[:, TPAD * h + 128 * tt:TPAD * h + 128 * (tt + 1)],
                                     wsb[:, coff + 512 * eo:coff + 512 * (eo + 1)],
                                     start=(h == 0), stop=(h == NH_T - 1))
                nc.scalar.copy(vsb[tt][:, 512 * eg:512 * (eg + 1)], pt[:])
                for _ in range(4):
                    if t1units:
                        t1units.pop(0)()
            if eg >= 1:
                av_heads(range(4 * (eg - 1), 4 * eg))
    while t1units:
        t1units.pop(0)()
    av_heads(range(12, HEADS))

    # ---- output projection: yT[h, t] = sum_e Wo[ed_e, h].T @ OT[ed_e, t] ----
    for ho in range(NH_T):
        yp = pst([128, TQ], F32, "yp")
        for e in range(HEADS):
            wsb = wt_wo[e // 2]
            coff = 2048 * (e % 2)
            nc.tensor.matmul(yp[:],
                             wsb[:, coff + 128 * ho:coff + 128 * (ho + 1)],
                             ot[e][:, 0:TQ],
                             start=(e == 0), stop=(e == HEADS - 1))
        ysb = apool.tile([128, TQ], F16, tag="y", name="ysb")
        nc.vector.tensor_copy(ysb[:], yp[:])
        (sy if ho >= 14 and ho % 2 == 0 else sc).dma_start(
            y_d[:, TQ * ho:TQ * (ho + 1)], ysb[:])


def _prep_inputs(x, mask, bias, Wq, Wk, Wv, Wo):
    """Slice/pad/transpose the full inputs into per-core prepacked maps."""
    x = np.asarray(x, np.float32).reshape(T, H)
    mask = np.asarray(mask, np.float32).reshape(T)
    bias = np.asarray(bias, np.float32).reshape(HEADS, T, T)

    # padded x (tokens) and mask, as in the reference
    xp = np.zeros((P, H), np.float32)
    xp[:T] = x
    mb = np.full(P, NEG, np.float32)
    mb[:T] = mask

    def mk_bias(qlo, qn_, klo):
        # [HEADS, KBLK, KBLK] combined bias for queries [qlo, qlo+qn_) vs
        # keys [klo, klo+KBLK). Rows >= qn_ fully masked. Pre-scaled by
        # sqrt(DH) (exp applies scale=DH**-0.5); masked entries use -100
        # (* SCALE -> -8.8, exp -> 1.4e-4: negligible vs real weights but
        # NONZERO so junk/pad row-sums stay finite - 1/0 = inf would turn
        # D's off-diagonal zeros into 0*inf = NaN, see _emit_body)
        blk = np.full((HEADS, KBLK, KBLK), -100.0, np.float32)
        qr = min(qn_, max(0, T - qlo))
        kr = min(KBLK, max(0, T - klo))
        if qr > 0 and kr > 0:
            b_ = bias[:, qlo:qlo + qr, klo:klo + kr] * (DH ** 0.5)
            pair = mb[qlo:qlo + qr, None] * mb[None, klo:klo + kr]
            blk[:, :qr, :kr] = b_ + np.where(pair > 0, 0.0, -100.0)[None]
        return blk

    # weight SBUF images: [128, 2g * 16h * (8e*128d)] for wq/wk/wv
    def pack_w(W):
        w = np.ascontiguousarray(W.reshape(H, H), np.float32).astype(np.float16)
        w = w.reshape(NH_T, 128, 2, 8 * 128)          # [h, p, g, ed]
        w = w.transpose(1, 2, 0, 3)                   # [p, g, h, ed]
        return np.ascontiguousarray(w.reshape(128, 2 * NH_T * 1024))

    wq = pack_w(Wq)
    wk = pack_w(Wk)
    wv = pack_w(Wv)
    # wo image: [128d, 16e * 2048h]
    wo = np.ascontiguousarray(Wo.reshape(HEADS, DH, H), np.float32).astype(np.float16)
    wo = wo.transpose(1, 0, 2)                        # [d, e, h]
    wo = np.ascontiguousarray(wo.reshape(128, HEADS * 2048))
    ident = np.eye(128, dtype=np.float16)

    def pack_x(xm, tl):
        xt = xm.T.astype(np.float16)              # [H, tl]
        xt = xt.reshape(NH_T, 128, tl).transpose(1, 0, 2)
        return np.ascontiguousarray(xt.reshape(128, NH_T * tl))

    in_maps = []
    for c in range(NCORES):
        qn = 7 if c < 2 else 6
        qoff = 7 * c if c < 2 else 14 + 6 * (c - 2)
        kblocks = [5 * c + j for j in range(5)] + [NB - 1]
        # query tokens: 5 own blocks + a qn-slice of block 40
        xqm = np.zeros((TQ, H), np.float32)
        # key/value tokens: the 6 kblocks (xk packed at 50, xv padded to 64)
        xkm = np.zeros((TPACK, H), np.float32)
        xvm = np.zeros((TPAD, H), np.float32)
        comb = np.zeros((NB_CORE, HEADS, KBLK, KBLK), np.float32)
        for j, b in enumerate(kblocks):
            lo = KBLK * b
            n = min(KBLK, max(0, T - lo))
            xkm[KBLK * j:KBLK * j + n] = xp[lo:lo + n]
            xvm[BPAD * j:BPAD * j + n] = xp[lo:lo + n]
            if j < 5:
                xqm[QCOL[j]:QCOL[j] + n] = xp[lo:lo + n]
                comb[j] = mk_bias(lo, KBLK, lo)
            else:
                qlo = KBLK * (NB - 1) + qoff
                nq = min(qn, max(0, T - qlo))
                xqm[QCOL[5]:QCOL[5] + nq] = xp[qlo:qlo + nq]
                comb[j] = mk_bias(qlo, qn, lo)

        # bias layout: [q, (block, headgroup, head, k)], fp16
        bc = comb.transpose(2, 0, 1, 3).reshape(KBLK, NB_CORE * HEADS * KBLK)
        in_maps.append({
            "xq": pack_x(xqm, TQ),
            "xk": pack_x(xkm, TPACK),
            "xv": pack_x(xvm, TPAD),
            "wq": wq, "wk": wk, "wv": wv, "wo": wo,
            "bias": np.ascontiguousarray(bc.astype(np.float16)),
            "ident": ident,
        })
    return in_maps


def _gather(results):
    out = np.empty((T, H), np.float32)
    for c in range(NCORES):
        # y is packed Y^T: [128, 16ho * 257] fp16
        yc = results[c]["y"].astype(np.float32)
        yc = yc.reshape(128, NH_T, TQ).transpose(1, 0, 2).reshape(H, TQ)
        for j in range(5):
            lo = KBLK * (5 * c + j)
            out[lo:lo + KBLK] = yc[:, QCOL[j]:QCOL[j] + KBLK].T
        qn = 7 if c < 2 else 6
        qoff = 7 * c if c < 2 else 14 + 6 * (c - 2)
        qlo = KBLK * (NB - 1) + qoff
        nq = min(qn, max(0, T - qlo))
        if nq > 0:
            out[qlo:qlo + nq] = yc[:, QCOL[5]:QCOL[5] + nq].T
    return out


def run(trace=False, **inputs):
    if "nc" not in _CACHE:
        _CACHE["nc"] = _build_program()
    nc = _CACHE["nc"]
    in_maps = _prep_inputs(
        inputs["x_BxTxH"], inputs["mask_BxT"], inputs["attention_bias_BxHxTxT"],
        inputs["Wq"], inputs["Wk"], inputs["Wv"], inputs["Wo"])
    res = run_bass_kernel_spmd(nc, in_maps, list(range(NCORES)), trace=trace)
    out = _gather(res.results)[None]       # restore batch dim [1, T, H]
    return out, res.exec_time_ns


def kernel(**inputs):
    out, _ = run(trace=False, **inputs)
    return out
